# revision 1
# baseline (speedup 1.0000x reference)
"""Multi-head attention (B=2, N=2048, D=1024, H=16) on 8 NeuronCores.

Sharding: data-parallel over batch (cores 0-3 -> b=0, cores 4-7 -> b=1),
tensor-parallel over heads (4 heads per core; column-parallel QKV,
row-parallel proj). Each core emits a partial projection output
y_c = O_heads(c) @ proj_w[rows(c)]; the host sums the 4 partials per batch
and adds proj_b.

Per-core kernel (Bass/Tile, fp32 data, float32r matmuls):
  A) PE-transpose x -> xT; qT/kT (head-pair-major) and v (n-major,
     ones-augmented column for the softmax denominator).
  B) flash-style attention in transposed space:
       ST[m,n] = kT.T qT  (PSUM) -> exp(SCALE*st) on ACT -> SBUF
       U = [v|1].T E accumulated over m-tiles; row 64 of U is the
       softmax denominator; normalize with a fast DVE copy (early PSUM
       release) + reciprocal + DRAM-bounce broadcast + multiply into
       OT (c-major).
  C) y = OT.T @ wp_local (K=256 over the core's head channels).
"""

import numpy as np

import concourse.bass as bass
import concourse.tile as tile
from concourse import mybir
from concourse.bass_utils import run_bass_kernel_spmd
from concourse.masks import make_identity
from concourse import library_config

# ---- problem constants (hardcoded per contract) ----
B = 2
N = 2048
D = 1024
H = 16
HD = 64          # head dim
SCALE = HD ** -0.5
NC = 8           # cores
HL = H // (NC // B)   # heads per core = 4
CW = HL * HD     # local qkv column width = 256

F32 = mybir.dt.float32
F32R = mybir.dt.float32r

MM_DT = F32R     # matmul compute dtype (bitcast view)

NT = N // 128    # 16 n-tiles (also m-tiles)
KC = D // 128    # 8 contraction chunks for qkv matmuls


def _mm(ap):
    """View an fp32 AP as the matmul compute dtype."""
    if MM_DT is F32:
        return ap
    return ap.bitcast(MM_DT)


def _r(ap):
    """Output-cast: write rounded to the matmul compute dtype (the BIR
    verifier requires fp32r matmul operands to be produced rounded)."""
    if MM_DT is F32:
        return ap
    return ap.bitcast(MM_DT)


def _split_sync_waits(nc, maxw: int = 1) -> int:
    """This walrus build rejects >1 semaphore-wait per instruction
    (setupSyncWait: "Too many sync wait commands"). Hoist excess waits
    onto preceding same-engine no-ops: the sequencer runs instructions
    in order, so the semantics are unchanged."""
    n_split = 0
    for fn in nc.m.functions:
        for bb in fn.blocks:
            insts = list(bb.instructions)
            out = []
            changed = False
            for inst in insts:
                si = inst.sync_info
                waits = list(si.on_wait) if si is not None and si.on_wait else []
                if len(waits) > maxw:
                    chunks = [waits[i: i + maxw] for i in range(0, len(waits), maxw)]
                    for chunk in chunks[:-1]:
                        out.append(mybir.InstNoOp(
                            name=f"I-splitw-{nc.next_id()}",
                            sync_info=mybir.SyncInfo(on_wait=chunk, on_update=[]),
                            bass_nofuse=True,
                            engine=inst.engine,
                        ))
                    si.on_wait = chunks[-1]
                    inst.sync_info = si
                    n_split += 1
                    changed = True
                out.append(inst)
            if changed:
                try:
                    bb.instructions = out
                except Exception:
                    bb.instructions.clear()
                    for i in out:
                        bb.instructions.append(i)
    return n_split


def _build_program(split=True, reps=1, stages="ABC"):
    nc = bass.Bass(trn_type="TRN2", target_bir_lowering=False, debug=False)

    x_d = nc.dram_tensor("x", [N, D], F32, kind="ExternalInput").ap()
    wq_d = nc.dram_tensor("wq", [D, CW], F32, kind="ExternalInput").ap()
    wk_d = nc.dram_tensor("wk", [D, CW], F32, kind="ExternalInput").ap()
    wv_d = nc.dram_tensor("wv", [D, CW], F32, kind="ExternalInput").ap()
    wp_d = nc.dram_tensor("wp", [CW, D], F32, kind="ExternalInput").ap()
    qkvb_d = nc.dram_tensor("qkvb", [3 * CW], F32, kind="ExternalInput").ap()
    y_d = nc.dram_tensor("y", [N, D], F32, kind="ExternalOutput").ap()

    with tile.TileContext(nc) as tc:
        for rep in range(reps):
            rsc_d = nc.dram_tensor(f"rscratch{rep}", [16, 512], F32).ap()
            _body(nc, tc, x_d, wq_d, wk_d, wv_d, wp_d, qkvb_d, y_d, rsc_d,
                  stages=stages)

    if split:
        _split_sync_waits(nc)
    return nc


def _body(nc, tc, x_d, wq_d, wk_d, wv_d, wp_d, qkvb_d, y_d, rsc_d, stages="ABC"):
    from contextlib import ExitStack

    persist = ExitStack()
    const_p = persist.enter_context(tc.tile_pool(name="const", bufs=1))
    qk_p = persist.enter_context(tc.tile_pool(name="qk", bufs=1))
    v1_p = persist.enter_context(tc.tile_pool(name="v1", bufs=1))

    ident = const_p.tile([128, 128], F32)
    make_identity(nc, ident)

    qT = qk_p.tile([128, 2, N], F32)      # [row-in-pair, pair, n]
    kT = qk_p.tile([128, 2, N], F32)
    v1 = v1_p.tile([128, NT, HL, HD + 1], F32)   # ones in last column

    qb = const_p.tile([128, 2], F32)
    kb = const_p.tile([128, 2], F32)
    vbc = const_p.tile([128, CW], F32)
    for pair in range(2):
        nc.gpsimd.dma_start(qb[:, pair: pair + 1],
                            qkvb_d[bass.ds(pair * 128, 128)].unsqueeze(1))
        nc.gpsimd.dma_start(kb[:, pair: pair + 1],
                            qkvb_d[bass.ds(CW + pair * 128, 128)].unsqueeze(1))
    nc.gpsimd.dma_start(
        vbc,
        qkvb_d[bass.ds(2 * CW, CW)].unsqueeze(0).partition_broadcast(128).squeeze(1))

    # ones column of v1 (DVE memset cannot emit f32r; use in0*0 + 1)
    nc.vector.tensor_scalar(
        _r(v1[:, :, :, HD]),
        vbc[:, 0:NT * HL].rearrange("p (a b) -> p a b", a=NT),
        0.0, 1.0, mybir.AluOpType.mult, mybir.AluOpType.add)

    # ---------------- Stage A pools (right side: freed mid-kernel) --------
    sa = ExitStack()    # w + xT: alive until the last qk matmul
    sa1 = ExitStack()   # x staging + wv + wraw: freed earlier
    w_p = sa.enter_context(tc.tile_pool(name="w", bufs=1, side="right"))
    xT_p = sa.enter_context(tc.tile_pool(name="xT", bufs=1, side="right"))
    wv_p = sa1.enter_context(tc.tile_pool(name="wv", bufs=1, side="right"))
    wraw_p = sa1.enter_context(tc.tile_pool(name="wraw", bufs=1, side="right"))
    xs_p = sa1.enter_context(tc.tile_pool(name="xs", bufs=9, side="right"))

    # stage-A PSUM pool (right side, freed with sa1)
    ps_a = sa1.enter_context(tc.tile_pool(name="ps_a", bufs=2, space="PSUM",
                                          side="right"))

    wq_s = w_p.tile([128, KC, CW], F32)
    wk_s = w_p.tile([128, KC, CW], F32)
    wv_s = wv_p.tile([128, KC, CW], F32)

    def load_weights():
        for (wd, ws) in ((wv_d, wv_s), (wq_d, wq_s), (wk_d, wk_s)):
            wr = wraw_p.tile([128, KC, CW], F32, tag="wraw", name="wraw")
            nc.gpsimd.dma_start(wr, wd.rearrange("(t p) c -> p t c", p=128))
            nc.vector.tensor_copy(_r(ws), wr)

    xT = xT_p.tile([128, KC, N], F32)

    def tg_load(g):
        """g indexes groups of 4 n-tiles (512 rows)."""
        xts = []
        for i in range(4):
            xt = xs_p.tile([128, D], F32, tag="xs", name="xs")
            nc.sync.dma_start(xt, x_d[bass.ds((g * 4 + i) * 128, 128), :])
            xts.append(xt)
        return xts

    def tg_dc(xts, g, dc):
        pt = ps_a.tile([128, 512], F32, tag="pt", name="pt")
        for i in range(4):
            nc.tensor.transpose(
                pt[:, i * 128:(i + 1) * 128],
                xts[i][:, dc * 128:(dc + 1) * 128],
                ident)
        nc.scalar.activation(
            _r(xT[:, dc, bass.ds(g * 512, 512)]), pt,
            mybir.ActivationFunctionType.Identity)

    def emit_v(mt):
        ps = ps_a.tile([128, CW], F32, tag="psv", name="psv")
        for dc in range(KC):
            nc.tensor.matmul(
                ps,
                _mm(xT[:, dc, bass.ds(mt * 128, 128)]),
                _mm(wv_s[:, dc, :]),
                start=(dc == 0), stop=(dc == KC - 1))
        nc.vector.tensor_add(
            _r(v1[:, mt, :, 0:HD]),
            ps.rearrange("p (h d) -> p h d", h=HL),
            vbc.rearrange("p (h d) -> p h d", h=HL))

    def emit_qk(pair, which, nb4):
        wt, dst, bias = ((wq_s, qT, qb), (wk_s, kT, kb))[which]
        ps = ps_a.tile([128, 512], F32, tag="psqk", name="psqk")
        for dc in range(KC):
            nc.tensor.matmul(
                ps,
                _mm(wt[:, dc, bass.ds(pair * 128, 128)]),
                _mm(xT[:, dc, bass.ds(nb4 * 512, 512)]),
                start=(dc == 0), stop=(dc == KC - 1))
        nc.scalar.activation(
            _r(dst[:, pair, bass.ds(nb4 * 512, 512)]), ps,
            mybir.ActivationFunctionType.Identity,
            bias=bias[:, pair: pair + 1])

    # --- prefix: what block(p0, nb0) mts 0..7 needs -----------------------
    xts01 = [tg_load(0), tg_load(1)]
    load_weights()
    for g in (0, 1):
        for dc in range(KC):
            tg_dc(xts01[g], g, dc)
    for mt in range(8):
        emit_v(mt)
    for nb4 in (0, 1):
        emit_qk(0, 0, nb4)
        emit_qk(0, 1, nb4)
    xts2 = tg_load(2)
    xts3 = tg_load(3)

    def emit_a_rest():
        for g, xts in ((2, xts2), (3, xts3)):
            for dc in range(KC):
                tg_dc(xts, g, dc)
        for mt in range(8, NT):
            emit_v(mt)
        for nb4 in (2, 3):
            emit_qk(0, 0, nb4)
            emit_qk(0, 1, nb4)
        for nb4 in range(4):
            emit_qk(1, 0, nb4)
            emit_qk(1, 1, nb4)

    if "B" not in stages:
        emit_a_rest()
        sa1.close()
        sa.close()
        persist.close()
        return

    # ---------------- Stage B (attention) + C (proj) ----------------------
    emit_a_rest()
    sa1.close()
    sa.close()

    sb = ExitStack()
    et_p = sb.enter_context(tc.tile_pool(name="et", bufs=6))
    ps_st = sb.enter_context(tc.tile_pool(name="ps_st", bufs=2, space="PSUM"))
    ps_u = sb.enter_context(tc.tile_pool(name="ps_u", bufs=1, space="PSUM"))

    def make_us():
        us = {}
        for sub in range(2):
            for jc in range(2):
                us[(sub, jc)] = ps_u.tile([HD + 1, 512], F32,
                                          tag=f"u{sub}{jc}",
                                          name=f"u_{sub}_{jc}")
        return us

    def emit_block_part(pair, nb, us, mts):
        for mt in mts:
            for sub in range(2):
                st = ps_st.tile([128, 1024], F32, tag="st", name="st")
                for jc in range(2):
                    nc.tensor.matmul(
                        st[:, jc * 512:(jc + 1) * 512],
                        _mm(kT[bass.ds(sub * HD, HD), pair,
                               bass.ds(mt * 128, 128)]),
                        _mm(qT[bass.ds(sub * HD, HD), pair,
                               bass.ds(nb * 1024 + jc * 512, 512)]),
                        start=True, stop=True)
                et = et_p.tile([128, 1024], F32, tag="et", name="et")
                nc.scalar.activation(
                    _r(et), st, mybir.ActivationFunctionType.Exp,
                    scale=float(SCALE))
                for jc in range(2):
                    nc.tensor.matmul(
                        us[(sub, jc)],
                        _mm(v1[:, mt, pair * 2 + sub, :]),
                        _mm(et[:, jc * 512:(jc + 1) * 512]),
                        start=(mt == 0), stop=(mt == NT - 1))

    def emit_norm_reads(pair, nb, us, ri_p, rb_p, otu_p):
        work = []
        for sub in range(2):
            for jc in range(2):
                u = us[(sub, jc)]
                idx = ((nb * 2) + pair) * 4 + sub * 2 + jc
                # read u out quickly so the PSUM slot frees for the next block
                otu = otu_p.tile([HD, 512], F32, tag="otu", name="otu")
                nc.vector.tensor_copy(otu, u[0:HD, :])
                ri = ri_p.tile([1, 512], F32, tag="ri", name="ri")
                nc.vector.reciprocal(ri, u[HD:HD + 1, :])
                nc.sync.dma_start(rsc_d[idx: idx + 1, :], ri)
                rb = rb_p.tile([HD, 512], F32, tag="rb", name="rb")
                nc.sync.dma_start(
                    rb,
                    rsc_d[idx, :].unsqueeze(0)
                    .partition_broadcast(HD).squeeze(1))
                work.append((sub, jc, otu, rb))
        return work

    def emit_norm_muls(pair, nb, work, OT, jcs=(0, 1)):
        for (sub, jc, otu, rb) in work:
            if jc not in jcs:
                continue
            nc.vector.tensor_mul(
                _r(OT[bass.ds(sub * HD, HD), pair,
                      bass.ds(nb * 1024 + jc * 512, 512)]),
                otu, rb)

    def emit_normalize(pair, nb, us, ri_p, rb_p, otu_p, OT):
        work = emit_norm_reads(pair, nb, us, ri_p, rb_p, otu_p)
        emit_norm_muls(pair, nb, work, OT)

    us00 = make_us()
    emit_block_part(0, 0, us00, range(0, NT))

    # late pools (fit after xT/w are freed)
    ot_p = sb.enter_context(tc.tile_pool(name="ot", bufs=1))
    OT = ot_p.tile([128, 2, N], F32)   # [c-in-pair, pair, n]
    ri_p = sb.enter_context(tc.tile_pool(name="ri", bufs=4))
    rb_p = sb.enter_context(tc.tile_pool(name="rb", bufs=6))
    otu_p = sb.enter_context(tc.tile_pool(name="otu", bufs=6))
    y_p = sb.enter_context(tc.tile_pool(name="y", bufs=4))
    wp_p = sb.enter_context(tc.tile_pool(name="wp", bufs=1))
    wp_s = wp_p.tile([128, 2, D], F32)
    wp_raw = wp_p.tile([128, 2, D], F32)
    nc.gpsimd.dma_start(wp_raw, wp_d.rearrange("(t p) e -> p t e", p=128))
    nc.vector.tensor_copy(_r(wp_s), wp_raw)

    emit_normalize(0, 0, us00, ri_p, rb_p, otu_p, OT)

    def emit_proj(nt):
        yt = y_p.tile([128, D], F32, tag="y", name="y")
        for ec in range(2):
            ps = ps_u.tile([128, 512], F32,
                           tag=f"u{nt % 2}{ec}",
                           name=f"psy_{nt}_{ec}")
            for pair in range(2):
                nc.tensor.matmul(
                    ps,
                    _mm(OT[:, pair, bass.ds(nt * 128, 128)]),
                    _mm(wp_s[:, pair, bass.ds(ec * 512, 512)]),
                    start=(pair == 0), stop=(pair == 1))
            nc.vector.tensor_copy(yt[:, bass.ds(ec * 512, 512)], ps)
        nc.sync.dma_start(y_d[bass.ds(nt * 128, 128), :], yt)

    for (pair, nb) in ((1, 0), (0, 1)):
        us = make_us()
        emit_block_part(pair, nb, us, range(NT))
        emit_normalize(pair, nb, us, ri_p, rb_p, otu_p, OT)
        if "C" in stages and (pair, nb) == (0, 1):
            for nt in range(0, NT // 2):
                emit_proj(nt)
    us = make_us()
    emit_block_part(1, 1, us, range(NT))
    work = emit_norm_reads(1, 1, us, ri_p, rb_p, otu_p)
    emit_norm_muls(1, 1, work, OT, jcs=(0,))
    if "C" in stages:
        for nt in range(NT // 2, NT // 2 + 4):
            emit_proj(nt)
    emit_norm_muls(1, 1, work, OT, jcs=(1,))
    if "C" in stages:
        for nt in range(NT // 2 + 4, NT):
            emit_proj(nt)

    sb.close()
    persist.close()


_NC_CACHE = None


def _get_program():
    global _NC_CACHE
    if _NC_CACHE is None:
        _NC_CACHE = _build_program()
    return _NC_CACHE


def make_in_maps(x, qkv_w, qkv_b, proj_w):
    in_maps = []
    for c in range(NC):
        b, j = divmod(c, NC // B)
        cs = j * CW
        in_maps.append({
            "x": np.ascontiguousarray(x[b], np.float32),
            "wq": np.ascontiguousarray(qkv_w[:, cs: cs + CW], np.float32),
            "wk": np.ascontiguousarray(qkv_w[:, D + cs: D + cs + CW], np.float32),
            "wv": np.ascontiguousarray(qkv_w[:, 2 * D + cs: 2 * D + cs + CW], np.float32),
            "wp": np.ascontiguousarray(proj_w[cs: cs + CW, :], np.float32),
            "qkvb": np.concatenate([
                qkv_b[cs: cs + CW],
                qkv_b[D + cs: D + cs + CW],
                qkv_b[2 * D + cs: 2 * D + cs + CW]]).astype(np.float32),
        })
    return in_maps


def combine_outputs(results, proj_b):
    out = np.empty((B, N, D), np.float32)
    per = NC // B
    for b in range(B):
        acc = results[b * per]["y"].astype(np.float32)
        for c in range(b * per + 1, (b + 1) * per):
            acc = acc + results[c]["y"]
        out[b] = acc + proj_b[None, :].astype(np.float32)
    return out


def kernel(**inputs):
    x = np.asarray(inputs["x"], np.float32)
    qkv_w = np.asarray(inputs["qkv_w"], np.float32)
    qkv_b = np.asarray(inputs["qkv_b"], np.float32)
    proj_w = np.asarray(inputs["proj_w"], np.float32)
    proj_b = np.asarray(inputs["proj_b"], np.float32)

    nc = _get_program()
    in_maps = make_in_maps(x, qkv_w, qkv_b, proj_w)
    res = run_bass_kernel_spmd(nc, in_maps, list(range(NC)), trace=False)
    return combine_outputs(res.results, proj_b)



# revision 4
# speedup vs baseline: 1.4053x; 1.4053x over previous
"""Multi-head attention (B=2, N=2048, D=1024, H=16) on 8 NeuronCores. v2.

Sharding: data-parallel over batch (cores 0-3 -> b=0, cores 4-7 -> b=1),
tensor-parallel over heads (4 heads per core; column-parallel QKV,
row-parallel proj). Each core emits a partial projection output
y_c = O_heads(c) @ proj_w[rows(c)]; the host sums the 4 partials per batch
and adds proj_b.

v2 design (vs v1):
  - Host supplies x pre-transposed + bf16 and weights pre-laid-out + bf16:
    no on-chip transposes, no staging pools, no weight casts.
  - All matmuls run bf16 (same PE rate as fp32r, half the SBUF).
  - Activation engine does ONLY the softmax exp (its serial floor);
    PSUM evacuation and bias adds ride on DVE.
  - Softmax normalization: reciprocal on DVE + PE broadcast matmul
    (ones[1,64] x ri[1,512]) instead of a DRAM round-trip.
  - Persistent tile pools with tag rotation so consecutive reps pipeline:
    qT/kT/v1 double-buffered; PSUM = 2x st[128,1024] + 4x u[65,512] tags.
  - QKV and proj work is woven into the attention blocks' PE stream to
    fill the PE slack while Act paces the exp pipeline.
"""

import numpy as np
import ml_dtypes

import concourse.bass as bass
import concourse.tile as tile
from concourse import mybir
from concourse.bass_utils import run_bass_kernel_spmd

# ---- problem constants (hardcoded per contract) ----
B = 2
N = 2048
D = 1024
H = 16
HD = 64          # head dim
SCALE = HD ** -0.5
NC = 8           # cores
HL = H // (NC // B)   # heads per core = 4
CW = HL * HD     # local qkv column width = 256

F32 = mybir.dt.float32
F32R = mybir.dt.float32r
BF16 = mybir.dt.bfloat16
NPBF16 = ml_dtypes.bfloat16

NT = N // 128    # 16 n-tiles (also m-tiles)
KC = D // 128    # 8 contraction chunks for qkv matmuls


def _r(ap):
    """f32r view of an fp32 AP (for the broadcast matmul operands)."""
    return ap.bitcast(F32R)


def _split_sync_waits(nc, maxw: int = 1) -> int:
    """This walrus build rejects >1 semaphore-wait per instruction
    (setupSyncWait: "Too many sync wait commands"). Hoist excess waits
    onto preceding same-engine no-ops: the sequencer runs instructions
    in order, so the semantics are unchanged."""
    n_split = 0
    for fn in nc.m.functions:
        for bb in fn.blocks:
            insts = list(bb.instructions)
            out = []
            changed = False
            for inst in insts:
                si = inst.sync_info
                waits = list(si.on_wait) if si is not None and si.on_wait else []
                if len(waits) > maxw:
                    chunks = [waits[i: i + maxw] for i in range(0, len(waits), maxw)]
                    for chunk in chunks[:-1]:
                        out.append(mybir.InstNoOp(
                            name=f"I-splitw-{nc.next_id()}",
                            sync_info=mybir.SyncInfo(on_wait=chunk, on_update=[]),
                            bass_nofuse=True,
                            engine=inst.engine,
                        ))
                    si.on_wait = chunks[-1]
                    inst.sync_info = si
                    n_split += 1
                    changed = True
                out.append(inst)
            if changed:
                try:
                    bb.instructions = out
                except Exception:
                    bb.instructions.clear()
                    for i in out:
                        bb.instructions.append(i)
    return n_split


def _build_program(split=True, reps=1):
    nc = bass.Bass(trn_type="TRN2", target_bir_lowering=False, debug=False)

    xt_d = nc.dram_tensor("xt", [4, 128, KC, 512], BF16, kind="ExternalInput").ap()
    wq_d = nc.dram_tensor("wq", [128, KC, CW], BF16, kind="ExternalInput").ap()
    wk_d = nc.dram_tensor("wk", [128, KC, CW], BF16, kind="ExternalInput").ap()
    wv_d = nc.dram_tensor("wv", [128, KC, CW], BF16, kind="ExternalInput").ap()
    wp_d = nc.dram_tensor("wp", [128, 2, D], BF16, kind="ExternalInput").ap()
    qb_d = nc.dram_tensor("qb", [128, 2], F32, kind="ExternalInput").ap()
    kb_d = nc.dram_tensor("kb", [128, 2], F32, kind="ExternalInput").ap()
    vb_d = nc.dram_tensor("vb", [CW], F32, kind="ExternalInput").ap()
    y_d = nc.dram_tensor("y", [N, D], F32, kind="ExternalOutput").ap()

    with tile.TileContext(nc) as tc:
        from contextlib import ExitStack
        persist = ExitStack()
        const_p = persist.enter_context(tc.tile_pool(name="const", bufs=1))
        data_p = persist.enter_context(tc.tile_pool(name="data", bufs=1))
        ps_p = persist.enter_context(
            tc.tile_pool(name="ps", bufs=1, space="PSUM"))

        # ---- one-time constants ----
        wq_s = const_p.tile([128, KC, CW], BF16)
        wk_s = const_p.tile([128, KC, CW], BF16)
        wv_s = const_p.tile([128, KC, CW], BF16)
        wp_s = const_p.tile([128, 2, D], BF16)
        qb_s = const_p.tile([128, 2], F32)
        kb_s = const_p.tile([128, 2], F32)
        vbc = const_p.tile([128, CW], F32)
        ones64 = const_p.tile([1, HD], F32)

        nc.gpsimd.dma_start(wq_s, wq_d)
        nc.gpsimd.dma_start(wk_s, wk_d)
        nc.gpsimd.dma_start(wv_s, wv_d)
        nc.gpsimd.dma_start(wp_s, wp_d)
        nc.gpsimd.dma_start(qb_s, qb_d)
        nc.gpsimd.dma_start(kb_s, kb_d)
        nc.gpsimd.dma_start(
            vbc,
            vb_d.unsqueeze(0).partition_broadcast(128).squeeze(1))
        # DVE memset cannot emit f32r; use in0*0 + 1 instead
        nc.vector.tensor_scalar(
            _r(ones64), vbc[0:1, 0:HD],
            0.0, 1.0, mybir.AluOpType.mult, mybir.AluOpType.add)

        for rep in range(reps):
            _body(nc, tc, const_p, data_p, ps_p,
                  xt_d, y_d, wq_s, wk_s, wv_s, wp_s, qb_s, kb_s, vbc, ones64)

        persist.close()

    if split:
        _split_sync_waits(nc)
    return nc


def _body(nc, tc, const_p, data_p, ps_p,
          xt_d, y_d, wq_s, wk_s, wv_s, wp_s, qb_s, kb_s, vbc, ones64):

    # ---- per-rep SBUF tiles (tag rotation gives cross-rep pipelining) ----
    xT = data_p.tile([128, KC, N], BF16, tag="xT", name="xT")
    qT = data_p.tile([128, 2, N], BF16, tag="qT", bufs=2, name="qT")
    kT = data_p.tile([128, 2, N], BF16, tag="kT", bufs=2, name="kT")
    v1 = data_p.tile([128, NT, HL, HD + 1], BF16, tag="v1", bufs=2, name="v1")
    OT = data_p.tile([128, 2, N], BF16, tag="OT", name="OT")

    # x arrives pre-transposed (bf16) in n-quarters
    for q in range(4):
        nc.sync.dma_start(xT[:, :, bass.ds(q * 512, 512)], xt_d[q])

    # ones column of v1 (in0*0 + 1)
    nc.vector.tensor_scalar(
        v1[:, :, :, HD],
        vbc[:, 0:NT * HL].rearrange("p (a b) -> p a b", a=NT),
        0.0, 1.0, mybir.AluOpType.mult, mybir.AluOpType.add)

    # ---------------- stage A emitters (PE + DVE) -------------------------
    def emit_vt_q(mt):
        """v for one m-tile -> v1 (n-major, with bias). ~0.85us PE lump."""
        vt = ps_p.tile([128, 256], F32, tag="st", bufs=3, name="vt")
        for dc in range(KC):
            nc.tensor.matmul(
                vt,
                xT[:, dc, bass.ds(mt * 128, 128)],
                wv_s[:, dc, :],
                start=(dc == 0), stop=(dc == KC - 1))
        nc.vector.tensor_add(
            v1[:, mt, :, 0:HD],
            vt.rearrange("p (h d) -> p h d", h=HL),
            vbc.rearrange("p (h d) -> p h d", h=HL))

    def emit_qk_half(p, which, h, nq):
        """q/k channels for pair p over one 512-n quarter (with bias)."""
        wt, dst, bias = ((wq_s, qT, qb_s), (wk_s, kT, kb_s))[which]
        t = ps_p.tile([128, 512], F32, tag="st", bufs=3, name="qkt")
        for dc in range(KC):
            nc.tensor.matmul(
                t,
                wt[:, dc, bass.ds(p * 128, 128)],
                xT[:, dc, bass.ds(h * 1024 + nq * 512, 512)],
                start=(dc == 0), stop=(dc == KC - 1))
        nc.vector.tensor_scalar_add(
            dst[:, p, bass.ds(h * 1024 + nq * 512, 512)], t, bias[:, p: p + 1])

    # ---------------- stage B/C emitters ----------------------------------
    def make_us():
        return {jc: ps_p.tile([HD + 1, 512], F32, tag=f"u{jc}",
                              name=f"u_{jc}")
                for jc in range(2)}

    def emit_st(pair, nb, mt, sub):
        """ST matmuls + exp for one (mt, sub) unit; returns the et tile."""
        st = ps_p.tile([128, 1024], F32, tag="st", bufs=3, name="st")
        for jc in range(2):
            nc.tensor.matmul(
                st[:, bass.ds(jc * 512, 512)],
                kT[bass.ds(sub * HD, HD), pair, bass.ds(mt * 128, 128)],
                qT[bass.ds(sub * HD, HD), pair,
                   bass.ds(nb * 1024 + jc * 512, 512)],
                start=True, stop=True)
        et = data_p.tile([128, 1024], BF16, tag="et", bufs=6, name="et")
        nc.scalar.activation(
            et, st, mybir.ActivationFunctionType.Exp, scale=float(SCALE))
        return et

    def emit_pv(pair, mt, sub, et, us):
        for jc in range(2):
            nc.tensor.matmul(
                us[jc],
                v1[:, mt, pair * 2 + sub, :],
                et[:, bass.ds(jc * 512, 512)],
                start=(mt == 0), stop=(mt == NT - 1))

    def emit_norm_reads(us, sub):
        """Evacuate one pass's U pair: numerators on Act, reciprocals on DVE,
        then the PE reciprocal-broadcast into the freed u slots."""
        work = []
        for jc in range(2):
            u = us[jc]
            otu = data_p.tile([HD, 512], F32, tag="otu", bufs=6, name="otu")
            nc.scalar.activation(
                otu, u[0:HD, :], mybir.ActivationFunctionType.Identity)
            ri = data_p.tile([1, 512], F32, tag="ri", bufs=4, name="ri")
            # f32r output is bit-identical to f32; the bitcast only marks the
            # data as rounded for the downstream f32r broadcast matmul.
            with nc.allow_low_precision(reason="f32r bitcast, same bits"):
                nc.vector.reciprocal(_r(ri), u[HD:HD + 1, :])
            rb = ps_p.tile([HD, 512], F32, tag=f"u{jc}", name="rb")
            nc.tensor.matmul(rb, _r(ones64), _r(ri), start=True, stop=True)
            work.append((sub, jc, otu, rb))
        return work

    def emit_norm_muls(pair, nb, work, jcs=(0, 1)):
        for (sub, jc, otu, rb) in work:
            if jc not in jcs:
                continue
            nc.vector.tensor_mul(
                OT[bass.ds(sub * HD, HD), pair,
                   bass.ds(nb * 1024 + jc * 512, 512)],
                otu, rb)

    def emit_proj(nt):
        yt = data_p.tile([128, D], F32, tag="y", bufs=4, name="y")
        for ec in range(2):
            pp = ps_p.tile([128, 512], F32, tag="st", bufs=3, name="pp")
            for pair in range(2):
                nc.tensor.matmul(
                    pp,
                    OT[:, pair, bass.ds(nt * 128, 128)],
                    wp_s[:, pair, bass.ds(ec * 512, 512)],
                    start=(pair == 0), stop=(pair == 1))
            nc.vector.tensor_copy(yt[:, bass.ds(ec * 512, 512)], pp)
        nc.gpsimd.dma_start(y_d[bass.ds(nt * 128, 128), :], yt)

    # ---------------- flat software-pipelined stream ----------------------
    # Sub-sequenced passes: each block runs all 16 m-tiles for sub 0, then
    # for sub 1, so only one jc-pair of U accumulators is live at a time
    # (2 PSUM banks) and the st rotation gets 3 slots (deeper Act backlog).
    # Position p emits PV(p-LAG), then ST(p)+exp, then woven lumps. A pass
    # tail triggers normalization inline, overlapping the next pass's head.
    LAG = 2
    blocks = [(0, 0), (1, 0), (0, 1), (1, 1)]
    passes = [(pair, nb, sub) for (pair, nb) in blocks for sub in range(2)]
    units = [(pair, nb, mt, sub)
             for (pair, nb, sub) in passes for mt in range(NT)]

    weave = {
        1: (lambda: emit_vt_q(2),),
        2: (lambda: emit_vt_q(3),),
        3: (lambda: (emit_vt_q(4), emit_qk_half(0, 1, 0, 1)),),
        4: (lambda: emit_vt_q(5),),
        5: (lambda: emit_vt_q(6),),
        6: (lambda: (emit_vt_q(7), emit_qk_half(0, 1, 1, 0)),),
        7: (lambda: emit_vt_q(8),),
        8: (lambda: emit_vt_q(9),),
        9: (lambda: emit_vt_q(10),),
        10: (lambda: (emit_vt_q(11), emit_qk_half(0, 1, 1, 1)),),
        11: (lambda: emit_vt_q(12),),
        12: (lambda: emit_vt_q(13),),
        13: (lambda: emit_vt_q(14),),
        14: (lambda: emit_vt_q(15),),
        17: (lambda: emit_qk_half(1, 0, 0, 0),),
        20: (lambda: emit_qk_half(1, 0, 0, 1),),
        23: (lambda: emit_qk_half(1, 1, 0, 0),),
        26: (lambda: emit_qk_half(1, 1, 0, 1),),
        29: (lambda: emit_qk_half(1, 1, 1, 0),),
        34: (lambda: emit_qk_half(1, 1, 1, 1),),
        39: (lambda: emit_qk_half(0, 0, 1, 0),),
        44: (lambda: emit_qk_half(0, 0, 1, 1),),
        50: (lambda: emit_qk_half(1, 0, 1, 0),),
        56: (lambda: emit_qk_half(1, 0, 1, 1),),
        66: (lambda: emit_proj(0),),
        70: (lambda: emit_proj(1),),
        74: (lambda: emit_proj(2),),
        78: (lambda: emit_proj(3),),
        84: (lambda: emit_proj(4),),
        90: (lambda: emit_proj(5),),
        98: (lambda: emit_proj(6),),
        106: (lambda: emit_proj(7),),
    }

    # prefix: minimum inputs for unit 0 (+LAG lookahead)
    emit_qk_half(0, 0, 0, 0)   # qT pair0 n 0..511
    emit_qk_half(0, 0, 0, 1)   # qT pair0 n 512..1023
    emit_qk_half(0, 1, 0, 0)   # kT pair0 m 0..511
    emit_vt_q(0)
    emit_vt_q(1)

    us_by_pass = {passes[0]: make_us()}
    ets = {}
    for p in range(len(units) + LAG):
        if p >= LAG:
            pair, nb, mt, sub = units[p - LAG]
            us = us_by_pass[(pair, nb, sub)]
            emit_pv(pair, mt, sub, ets.pop(p - LAG), us)
            if mt == NT - 1:
                work = emit_norm_reads(us, sub)
                pidx = passes.index((pair, nb, sub))
                last = pidx == len(passes) - 1
                if not last:
                    us_by_pass[passes[pidx + 1]] = make_us()
                    emit_norm_muls(pair, nb, work)
                else:
                    emit_norm_muls(pair, nb, work, jcs=(0,))
                    for nt in range(8, 12):
                        emit_proj(nt)
                    emit_norm_muls(pair, nb, work, jcs=(1,))
                    for nt in range(12, NT):
                        emit_proj(nt)
        if p < len(units):
            pair, nb, mt, sub = units[p]
            ets[p] = emit_st(pair, nb, mt, sub)
        for fn in weave.get(p, ()):
            fn()


_NC_CACHE = None


def _get_program():
    global _NC_CACHE
    if _NC_CACHE is None:
        _NC_CACHE = _build_program()
    return _NC_CACHE


def make_in_maps(x, qkv_w, qkv_b, proj_w):
    in_maps = []
    for c in range(NC):
        b, j = divmod(c, NC // B)
        cs = j * CW

        # x^T in bf16, n-quarter-major: [4, 128, KC, 512]
        xt = np.ascontiguousarray(x[b].T.astype(NPBF16))       # [D, N]
        xt = xt.reshape(KC, 128, 4, 512)                        # dc, dpart, q, n
        xt = np.ascontiguousarray(xt.transpose(2, 1, 0, 3))     # q, dpart, dc, n

        def wlay(w):   # [D, CW] -> [128, KC, CW] bf16
            return np.ascontiguousarray(
                w.reshape(KC, 128, CW).transpose(1, 0, 2).astype(NPBF16))

        wp_l = proj_w[cs: cs + CW, :]                           # [CW, D]
        wp = np.ascontiguousarray(
            wp_l.reshape(2, 128, D).transpose(1, 0, 2).astype(NPBF16))

        def blay(bvec):  # [CW] -> [128, 2] (pair-major channels)
            return np.ascontiguousarray(
                bvec.reshape(2, 128).T.astype(np.float32))

        in_maps.append({
            "xt": xt,
            "wq": wlay(qkv_w[:, cs: cs + CW]),
            "wk": wlay(qkv_w[:, D + cs: D + cs + CW]),
            "wv": wlay(qkv_w[:, 2 * D + cs: 2 * D + cs + CW]),
            "wp": wp,
            "qb": blay(qkv_b[cs: cs + CW]),
            "kb": blay(qkv_b[D + cs: D + cs + CW]),
            "vb": np.ascontiguousarray(
                qkv_b[2 * D + cs: 2 * D + cs + CW].astype(np.float32)),
        })
    return in_maps


def combine_outputs(results, proj_b):
    out = np.empty((B, N, D), np.float32)
    per = NC // B
    for b in range(B):
        acc = results[b * per]["y"].astype(np.float32)
        for c in range(b * per + 1, (b + 1) * per):
            acc = acc + results[c]["y"]
        out[b] = acc + proj_b[None, :].astype(np.float32)
    return out


def kernel(**inputs):
    x = np.asarray(inputs["x"], np.float32)
    qkv_w = np.asarray(inputs["qkv_w"], np.float32)
    qkv_b = np.asarray(inputs["qkv_b"], np.float32)
    proj_w = np.asarray(inputs["proj_w"], np.float32)
    proj_b = np.asarray(inputs["proj_b"], np.float32)

    nc = _get_program()
    in_maps = make_in_maps(x, qkv_w, qkv_b, proj_w)
    res = run_bass_kernel_spmd(nc, in_maps, list(range(NC)), trace=False)
    return combine_outputs(res.results, proj_b)
